# revision 1
# baseline (speedup 1.0000x reference)
"""Location-sensitive attention Trainium2 kernel.

Strategy (data-parallel over batch, 8 cores, B=128 -> 16 per core):
  - Host pre-marshals everything the PE needs so the device does ZERO
    transposes of the big tensor: encoder shipped in BOTH layouts as bf16
    (natural [B*T, E] for the context matmul, transposed [E, B*T] for the
    projection matmul). Total HBM per core = 16.8 MB, same bytes as one
    fp32 copy.
  - conv1d folded into W_loc on the host: loc_p = (W_loc @ conv_w) applied
    to a host-built im2col of prev_attention -> accumulates into the same
    PSUM tile as the encoder projection.
  - decoder projection applied as a per-partition bias inside the tanh
    activation.
  - energies via a per-batch column-masked W_e ("diag" trick) so all 16
    batch tiles accumulate into one [16, 512] PSUM tile (softmax layout,
    no gather copies).
  - b_e dropped: softmax is shift-invariant, so it cannot affect outputs.
"""

import sys

for p in ("/opt/trn_rl_repo",):
    if p not in sys.path:
        sys.path.insert(0, p)

import numpy as np
import ml_dtypes

import concourse.bass as bass
import concourse.tile as tile
from concourse import mybir
from concourse import bacc
from concourse import bass_utils
from concourse.masks import make_identity

BF = ml_dtypes.bfloat16

NCORES = 8
B, T, E, D, A, F, KW = 128, 512, 512, 1024, 128, 32, 31
BS = B // NCORES          # 16 batches per core
NT = BS                   # 16 bt-tiles of 512 (tile i == batch i)
CH = BS * T // 128        # 64 chunks of 128 rows
P = 128


def build_device_program(nc):
    dt = mybir.dt
    f32, bf16 = dt.float32, dt.bfloat16
    Act = mybir.ActivationFunctionType

    # Both encoder layouts arrive partition-major ([p, ...] with each
    # partition's data contiguous in DRAM) so every DMA descriptor is one
    # large contiguous run.
    enc_nat = nc.dram_tensor("enc_nat", (P, CH, E), bf16, kind="ExternalInput").ap()
    encT = nc.dram_tensor("encT", (P, 4, BS * T), bf16, kind="ExternalInput").ap()
    prevrep = nc.dram_tensor("prevrep", (32, BS * T), bf16, kind="ExternalInput").ap()
    decT = nc.dram_tensor("decT", (D, BS), f32, kind="ExternalInput").ap()
    w_encT = nc.dram_tensor("w_encT", (E, A), bf16, kind="ExternalInput").ap()
    w_decT = nc.dram_tensor("w_decT", (D, A), f32, kind="ExternalInput").ap()
    w_combT = nc.dram_tensor("w_combT", (32, A), bf16, kind="ExternalInput").ap()
    w_ediag = nc.dram_tensor("w_ediag", (A, BS * BS), bf16, kind="ExternalInput").ap()
    ctx_out = nc.dram_tensor("context_out", (BS, E), f32, kind="ExternalOutput").ap()
    attn_out = nc.dram_tensor("attn_out", (BS, T), f32, kind="ExternalOutput").ap()

    with tile.TileContext(nc) as tc:
        with (
            tc.tile_pool(name="const", bufs=1) as const,
            tc.tile_pool(name="big", bufs=1) as big,
            tc.tile_pool(name="work", bufs=1) as work,
            tc.tile_pool(name="ps_pe", bufs=2, space="PSUM") as ps_pe,
            tc.tile_pool(name="ps_one", bufs=1, space="PSUM") as ps_one,
            tc.tile_pool(name="ps_sm", bufs=2, space="PSUM") as ps_sm,
        ):
            # ---- small constants / weights ----
            w_encT_sb = const.tile([P, 4, A], bf16)
            nc.sync.dma_start(w_encT_sb, w_encT.rearrange("(o p) a -> p o a", p=P))
            w_decT_sb = const.tile([P, 8, A], f32)
            nc.sync.dma_start(w_decT_sb, w_decT.rearrange("(o p) a -> p o a", p=P))
            w_combT_sb = const.tile([32, A], bf16)
            nc.sync.dma_start(w_combT_sb, w_combT)
            w_ediag_sb = const.tile([A, BS * BS], bf16)
            nc.sync.dma_start(w_ediag_sb, w_ediag)
            decT_sb = const.tile([P, 8, BS], f32)
            nc.sync.dma_start(decT_sb, decT.rearrange("(o p) b -> p o b", p=P))
            prevrep_sb = const.tile([32, BS * T], bf16)
            nc.sync.dma_start(prevrep_sb, prevrep)
            ident16 = const.tile([16, 16], f32)
            make_identity(nc, ident16)

            # ---- big loads (contiguous per partition; ~1MB each) ----
            # encT feeds the projection phase first: issue on sync.
            encT_sb = big.tile([P, 4, BS * T], bf16)
            HALF = BS * T // 2
            for et in range(4):
                for h in range(2):
                    nc.sync.dma_start(
                        encT_sb[:, et, h * HALF:(h + 1) * HALF],
                        encT[:, et, h * HALF:(h + 1) * HALF],
                    )
            # enc natural is only needed for the (late) context phase: issue
            # on the second HWDGE engine (scalar) so descriptor generation
            # doesn't serialize behind encT's.
            enc_nat_sb = big.tile([P, CH, E], bf16)
            for h in range(8):
                nc.sync.dma_start(
                    enc_nat_sb[:, h * 8:(h + 1) * 8, :], enc_nat[:, h * 8:(h + 1) * 8, :]
                )

            # ---- dec_p: [A, BS] = W_decT.T @ decT ----
            psum_dec = ps_one.tile([A, BS], f32, tag="dec")
            for dti in range(8):
                nc.tensor.matmul(
                    psum_dec,
                    lhsT=w_decT_sb[:, dti, :],
                    rhs=decT_sb[:, dti, :],
                    start=(dti == 0),
                    stop=(dti == 7),
                )
            decp_sb = const.tile([A, BS], f32)
            nc.vector.tensor_copy(decp_sb, psum_dec)

            # ---- projection + tanh + energies ----
            psum_energ = ps_one.tile([BS, T], f32, tag="energ")
            tanh_tiles = []
            for i in range(NT):  # tile i == batch i
                pe_t = ps_pe.tile([A, T], f32, tag="pe")
                for et in range(4):
                    nc.tensor.matmul(
                        pe_t,
                        lhsT=w_encT_sb[:, et, :],
                        rhs=encT_sb[:, et, i * T:(i + 1) * T],
                        start=(et == 0),
                        stop=False,
                    )
                nc.tensor.matmul(
                    pe_t,
                    lhsT=w_combT_sb,
                    rhs=prevrep_sb[:, i * T:(i + 1) * T],
                    start=False,
                    stop=True,
                )
                tanh_t = work.tile([A, T], bf16, tag="tanh", bufs=4)
                nc.scalar.activation(
                    tanh_t, pe_t, Act.Tanh, bias=decp_sb[:, i:i + 1], scale=1.0
                )
                tanh_tiles.append(tanh_t)
                nc.tensor.matmul(
                    psum_energ,
                    lhsT=w_ediag_sb[:, i * BS:(i + 1) * BS],
                    rhs=tanh_t,
                    start=(i == 0),
                    stop=(i == NT - 1),
                )

            # ---- softmax over T (psum_energ is [16, 512]) ----
            negmx = work.tile([BS, 1], f32)
            nc.vector.tensor_reduce(
                negmx, psum_energ, axis=mybir.AxisListType.X,
                op=mybir.AluOpType.max, negate=True,
            )
            attn_exp = work.tile([BS, T], f32)
            esum = work.tile([BS, 1], f32)
            nc.scalar.activation(
                attn_exp, psum_energ, Act.Exp, bias=negmx, scale=1.0, accum_out=esum
            )
            rs = work.tile([BS, 1], f32)
            nc.vector.reciprocal(rs, esum)
            attn_f32 = work.tile([BS, T], f32)
            nc.vector.tensor_scalar_mul(attn_f32, attn_exp, rs)
            nc.sync.dma_start(attn_out, attn_f32)

            # ---- transpose attn -> [t, b] columns ----
            attnT_sb = work.tile([P, 4, BS], bf16)
            for j in range(4):
                ps_t = ps_sm.tile([P, BS], f32, tag="attnT")
                nc.tensor.transpose(ps_t, attn_f32[:, j * P:(j + 1) * P], ident16)
                nc.vector.tensor_copy(attnT_sb[:, j, :], ps_t)

            # ---- context ----
            # M=1 matmuls with offset weight columns are miscompiled by this
            # walrus (only offset-0 lhsT works), so use the full [128,16]
            # attnT block as lhsT: out[b', :] = sum_t attnT[t, b'] enc[(b,t), :].
            # Only row b is the true context for batch b (other rows mix
            # batches); extract it with a per-row DMA.
            for b in range(BS):
                pc = ps_sm.tile([BS, E], f32, tag="ctx")
                for j in range(4):
                    nc.tensor.matmul(
                        pc,
                        lhsT=attnT_sb[:, j, :],
                        rhs=enc_nat_sb[:, 4 * b + j, :],
                        start=(j == 0),
                        stop=(j == 3),
                    )
                ctxg = work.tile([BS, E], f32, tag="ctxg", bufs=2)
                if b % 2 == 0:
                    nc.vector.tensor_copy(ctxg, pc)
                else:
                    nc.scalar.copy(ctxg, pc)
                nc.sync.dma_start(ctx_out[b:b + 1, :], ctxg[b:b + 1, :])

    return nc


def host_prepare(encoder_outputs, decoder_state, prev_attention_weights,
                 W_enc, W_dec, conv_w, W_loc, W_e, b_e):
    """Build per-core input maps (host-side marshaling, all numpy)."""
    f32 = np.float32
    enc = np.asarray(encoder_outputs, dtype=f32)
    dec = np.asarray(decoder_state, dtype=f32)
    prev = np.asarray(prev_attention_weights, dtype=f32)
    W_enc = np.asarray(W_enc, dtype=f32)
    W_dec = np.asarray(W_dec, dtype=f32)
    conv_w = np.asarray(conv_w, dtype=f32)
    W_loc = np.asarray(W_loc, dtype=f32)
    W_e = np.asarray(W_e, dtype=f32)

    w_encT = np.ascontiguousarray(W_enc.T).astype(BF)          # [E, A]
    w_decT = np.ascontiguousarray(W_dec.T)                     # [D, A] f32
    Wcomb = W_loc @ conv_w[:, 0, :]                            # [A, KW]
    w_combT = np.zeros((32, A), dtype=BF)
    w_combT[:KW] = Wcomb.T.astype(BF)
    w_ediag = np.zeros((A, BS * BS), dtype=BF)
    we = W_e[0].astype(BF)                                     # [A]
    for b in range(BS):
        w_ediag[:, b * BS + b] = we

    pp = np.pad(prev, ((0, 0), (15, 15)))                      # [B, T+30]

    in_maps = []
    for c in range(NCORES):
        sl = slice(c * BS, (c + 1) * BS)
        enc_c = enc[sl].reshape(BS * T, E).astype(BF)
        # partition-major natural layout: [p, chunk, e]
        enc_nat = np.ascontiguousarray(
            enc_c.reshape(BS * T // 128, 128, E).transpose(1, 0, 2)
        )
        # partition-major transposed layout: [p, e_tile, bt]
        encT = np.ascontiguousarray(
            enc_c.T.reshape(4, 128, BS * T).transpose(1, 0, 2)
        )
        rep = np.zeros((32, BS, T), dtype=BF)
        pc = pp[sl]
        for k in range(KW):
            rep[k] = pc[:, k:k + T].astype(BF)
        in_maps.append({
            "enc_nat": enc_nat,
            "encT": encT,
            "prevrep": np.ascontiguousarray(rep.reshape(32, BS * T)),
            "decT": np.ascontiguousarray(dec[sl].T),           # [D, BS] f32
            "w_encT": w_encT,
            "w_decT": w_decT,
            "w_combT": w_combT,
            "w_ediag": w_ediag,
        })
    return in_maps


_NC_CACHE = {}


def get_nc():
    if "nc" not in _NC_CACHE:
        nc = bacc.Bacc("TRN2", debug=False, num_devices=NCORES)
        build_device_program(nc)
        nc.finalize()
        _NC_CACHE["nc"] = nc
    return _NC_CACHE["nc"]


def kernel(encoder_outputs, decoder_state, prev_attention_weights,
           W_enc, W_dec, conv_w, W_loc, W_e, b_e, _trace=False, _result_box=None):
    in_maps = host_prepare(
        encoder_outputs, decoder_state, prev_attention_weights,
        W_enc, W_dec, conv_w, W_loc, W_e, b_e,
    )
    nc = get_nc()
    res = bass_utils.run_bass_kernel_spmd(
        nc, in_maps, core_ids=list(range(NCORES)), trace=_trace,
    )
    if _result_box is not None:
        _result_box.append(res)
    ctx = np.concatenate([r["context_out"] for r in res.results], axis=0)
    attn = np.concatenate([r["attn_out"] for r in res.results], axis=0)
    return ctx.astype(np.float32), attn.astype(np.float32)



# revision 4
# speedup vs baseline: 1.2273x; 1.2273x over previous
"""Location-sensitive attention Trainium2 kernel (v2 — stream-chasing).

Strategy (data-parallel over batch, 8 cores, B=128 -> 16 per core):
  - encoder shipped in BOTH layouts as bf16 (transposed [E, bt] for the
    projection, natural [bt, E] for the context) — same total HBM bytes
    as one fp32 copy.  Both are partition-major with >=8KB contiguous
    per-partition runs.
  - encT arrives in 8 batch-major tiles so projection matmuls chase the
    DMA stream instead of waiting for the whole tensor.
  - enc_nat loads are gated behind encT completion via tiny WAW "gate"
    DMAs, so the two streams don't split DMA bandwidth while the
    projection is the critical consumer.
  - conv1d folded into W_loc on the host (im2col prevrep), accumulated
    into the same PSUM tile as the encoder projection.
  - decoder projection applied as a per-partition bias inside tanh.
  - energies via per-batch column-masked W_e ("diag" trick) into one
    [16, 512] PSUM tile.
  - context: block-diagonal scattered attn-transpose tile L so all 64
    (batch, t-chunk) matmuls accumulate into ONE [16, E] PSUM tile;
    softmax normalization folded into the final copy's per-row scale.
  - b_e dropped: softmax is shift-invariant.
"""

import sys

for p in ("/opt/trn_rl_repo",):
    if p not in sys.path:
        sys.path.insert(0, p)

import numpy as np
import ml_dtypes

import concourse.bass as bass
import concourse.tile as tile
from concourse import mybir
from concourse import bacc
from concourse import bass_utils
from concourse.masks import make_identity

BF = ml_dtypes.bfloat16

NCORES = 8
B, T, E, D, A, F, KW = 128, 512, 512, 1024, 128, 32, 31
BS = B // NCORES          # 16 batches per core
NG = 8                    # encT / enc_nat arrive in 8 tiles of 2 batches
P = 128


def build_device_program(nc):
    dt = mybir.dt
    f32, bf16 = dt.float32, dt.bfloat16
    Act = mybir.ActivationFunctionType

    # All DRAM layouts are partition-major with large contiguous
    # per-partition runs (8-16 KB descriptors).
    encT = nc.dram_tensor("encT", (P, NG, 2, 4, T), bf16, kind="ExternalInput").ap()
    enc_nat = nc.dram_tensor("enc_nat", (P, NG, 8, E), bf16, kind="ExternalInput").ap()
    prevrep = nc.dram_tensor("prevrep", (32, BS * T), bf16, kind="ExternalInput").ap()
    decT = nc.dram_tensor("decT", (D, BS), bf16, kind="ExternalInput").ap()
    w_encT = nc.dram_tensor("w_encT", (E, A), bf16, kind="ExternalInput").ap()
    w_decT = nc.dram_tensor("w_decT", (D, A), bf16, kind="ExternalInput").ap()
    w_combT = nc.dram_tensor("w_combT", (32, A), bf16, kind="ExternalInput").ap()
    w_ediag = nc.dram_tensor("w_ediag", (A, BS * BS), bf16, kind="ExternalInput").ap()
    ctx_out = nc.dram_tensor("context_out", (BS, E), f32, kind="ExternalOutput").ap()
    attn_out = nc.dram_tensor("attn_out", (BS, T), f32, kind="ExternalOutput").ap()

    with tile.TileContext(nc) as tc:
        with (
            tc.tile_pool(name="const", bufs=1) as const,
            tc.tile_pool(name="big", bufs=1) as big,
            tc.tile_pool(name="work", bufs=1) as work,
            tc.tile_pool(name="ps_pe", bufs=3, space="PSUM") as ps_pe,
            tc.tile_pool(name="ps_one", bufs=1, space="PSUM") as ps_one,
        ):
            # ---- small constants / weights on the SWDGE queue so the
            # sync HWDGE queue is free for the encoder stream ----
            w_encT_sb = const.tile([P, 4, A], bf16)
            nc.gpsimd.dma_start(w_encT_sb, w_encT.rearrange("(o p) a -> p o a", p=P))
            w_combT_sb = const.tile([32, A], bf16)
            nc.gpsimd.dma_start(w_combT_sb, w_combT)
            prevrep_sb = const.tile([32, BS * T], bf16)
            nc.gpsimd.dma_start(prevrep_sb, prevrep)
            w_decT_sb = const.tile([P, 8, A], bf16)
            nc.gpsimd.dma_start(w_decT_sb, w_decT.rearrange("(o p) a -> p o a", p=P))
            decT_sb = const.tile([P, 8, BS], bf16)
            nc.gpsimd.dma_start(decT_sb, decT.rearrange("(o p) b -> p o b", p=P))
            w_ediag_sb = const.tile([A, BS * BS], bf16)
            nc.gpsimd.dma_start(w_ediag_sb, w_ediag)
            ident16 = const.tile([16, 16], bf16)
            make_identity(nc, ident16)

            # L: block-diagonal scattered attn-transpose, zeroed early
            # (no deps); the 16 nonzero column-strips are filled after
            # softmax.
            L = work.tile([P, 4 * BS, BS], bf16)
            nc.vector.memset(L, 0.0)

            # ---- encT stream: 8 batch-major tiles on sync HWDGE ----
            encT_sb = [big.tile([P, 2, 4, T], bf16, name=f"encT{g}", tag=f"encT{g}")
                       for g in range(NG)]
            for g in range(NG):
                nc.sync.dma_start(encT_sb[g], encT[:, g])

            # ---- enc_nat stream, gated behind encT tile 6 ----
            # The scheduler is a ready-heap, so ordering must be a real
            # dependency: each gate reads encT tile 6 and writes a corner
            # of the nat tile (WAW with the big load that follows).
            nat_sb = [big.tile([P, 8, E], bf16, name=f"nat{g}", tag=f"nat{g}")
                      for g in range(NG)]
            for g in range(NG):
                nc.sync.dma_start(nat_sb[g][0:1, 0, 0:64], encT_sb[6][0:1, 0, 0, 0:64])
                nc.sync.dma_start(nat_sb[g], enc_nat[:, g])

            # ---- dec_p: [A, BS] = W_decT.T @ decT ----
            psum_dec = ps_one.tile([A, BS], f32, tag="dec")
            for i in range(8):
                nc.tensor.matmul(
                    psum_dec,
                    lhsT=w_decT_sb[:, i, :],
                    rhs=decT_sb[:, i, :],
                    start=(i == 0),
                    stop=(i == 7),
                )
            decp_sb = const.tile([A, BS], f32)
            nc.vector.tensor_copy(decp_sb, psum_dec)

            # ---- projection + tanh + energies, chasing the encT stream ----
            psum_energ = ps_one.tile([BS, T], f32, tag="energ")
            for b in range(BS):
                g, j = b // 2, b % 2
                pe_t = ps_pe.tile([A, T], f32, tag="pe")
                for et in range(4):
                    nc.tensor.matmul(
                        pe_t,
                        lhsT=w_encT_sb[:, et, :],
                        rhs=encT_sb[g][:, j, et, :],
                        start=(et == 0),
                        stop=False,
                    )
                nc.tensor.matmul(
                    pe_t,
                    lhsT=w_combT_sb,
                    rhs=prevrep_sb[:, b * T:(b + 1) * T],
                    start=False,
                    stop=True,
                )
                tanh_t = work.tile([A, T], bf16, tag="tanh", bufs=4)
                nc.scalar.activation(
                    tanh_t, pe_t, Act.Tanh, bias=decp_sb[:, b:b + 1], scale=1.0
                )
                nc.tensor.matmul(
                    psum_energ,
                    lhsT=w_ediag_sb[:, b * BS:(b + 1) * BS],
                    rhs=tanh_t,
                    start=(b == 0),
                    stop=(b == BS - 1),
                )

            # ---- softmax over T (psum_energ is [16, 512]) ----
            negmx = work.tile([BS, 1], f32)
            nc.vector.tensor_reduce(
                negmx, psum_energ, axis=mybir.AxisListType.X,
                op=mybir.AluOpType.max, negate=True,
            )
            # Unnormalized exp in bf16 feeds the context path; the 1/sum
            # is folded into the final context copy's per-row scale.
            exp_bf = work.tile([BS, T], bf16)
            esum = work.tile([BS, 1], f32)
            nc.scalar.activation(
                exp_bf, psum_energ, Act.Exp, bias=negmx, scale=1.0, accum_out=esum
            )
            rs = work.tile([BS, 1], f32)
            nc.vector.reciprocal(rs, esum)
            attn_f32 = work.tile([BS, T], f32)
            nc.vector.tensor_scalar_mul(attn_f32, exp_bf, rs)
            nc.sync.dma_start(attn_out, attn_f32)

            # ---- exp^T -> block-diagonal L ----
            psum_at = ps_one.tile([P, 4, BS], bf16, tag="attnT")
            for q in range(4):
                nc.tensor.transpose(
                    psum_at[:, q, :], exp_bf[:, q * P:(q + 1) * P], ident16
                )
            for b in range(BS):
                nc.vector.tensor_copy(
                    L[:, 4 * b:4 * b + 4, b:b + 1], psum_at[:, :, b:b + 1]
                )

            # ---- context: 64 chunk matmuls into ONE [16, E] psum ----
            psum_ctx = ps_one.tile([BS, E], f32, tag="ctx")
            for g in range(NG):
                for k in range(8):
                    c = 8 * g + k
                    nc.tensor.matmul(
                        psum_ctx,
                        lhsT=L[:, c, :],
                        rhs=nat_sb[g][:, k, :],
                        start=(c == 0),
                        stop=(c == 63),
                    )
            ctxg = work.tile([BS, E], f32)
            nc.scalar.activation(ctxg, psum_ctx, Act.Copy, scale=rs)
            nc.sync.dma_start(ctx_out, ctxg)

    return nc


def host_prepare(encoder_outputs, decoder_state, prev_attention_weights,
                 W_enc, W_dec, conv_w, W_loc, W_e, b_e):
    """Build per-core input maps (host-side marshaling, all numpy)."""
    f32 = np.float32
    enc = np.asarray(encoder_outputs, dtype=f32)
    dec = np.asarray(decoder_state, dtype=f32)
    prev = np.asarray(prev_attention_weights, dtype=f32)
    W_enc = np.asarray(W_enc, dtype=f32)
    W_dec = np.asarray(W_dec, dtype=f32)
    conv_w = np.asarray(conv_w, dtype=f32)
    W_loc = np.asarray(W_loc, dtype=f32)
    W_e = np.asarray(W_e, dtype=f32)

    w_encT = np.ascontiguousarray(W_enc.T).astype(BF)          # [E, A]
    w_decT = np.ascontiguousarray(W_dec.T).astype(BF)          # [D, A]
    Wcomb = W_loc @ conv_w[:, 0, :]                            # [A, KW]
    w_combT = np.zeros((32, A), dtype=BF)
    w_combT[:KW] = Wcomb.T.astype(BF)
    w_ediag = np.zeros((A, BS * BS), dtype=BF)
    we = W_e[0].astype(BF)                                     # [A]
    for b in range(BS):
        w_ediag[:, b * BS + b] = we

    pp = np.pad(prev, ((0, 0), (15, 15)))                      # [B, T+30]

    in_maps = []
    for c in range(NCORES):
        sl = slice(c * BS, (c + 1) * BS)
        enc_c = enc[sl].astype(BF)                             # [BS, T, E]
        # encT: [p, g, j, et, t] = enc[2g+j, t, et*128+p]
        encT = np.ascontiguousarray(
            enc_c.transpose(2, 0, 1)                           # [E, BS, T]
            .reshape(4, P, NG, 2, T)
            .transpose(1, 2, 3, 0, 4)                          # [p, g, j, et, t]
        )
        # enc_nat: [p, g, k, e] = enc[b, q*128+p, e],  8g+k = 4b+q
        enc_nat = np.ascontiguousarray(
            enc_c.reshape(BS * 4, P, E).transpose(1, 0, 2)     # [p, 64, E]
            .reshape(P, NG, 8, E)
        )
        rep = np.zeros((32, BS, T), dtype=BF)
        pc = pp[sl]
        for k in range(KW):
            rep[k] = pc[:, k:k + T].astype(BF)
        in_maps.append({
            "encT": encT,
            "enc_nat": enc_nat,
            "prevrep": np.ascontiguousarray(rep.reshape(32, BS * T)),
            "decT": np.ascontiguousarray(dec[sl].T).astype(BF),  # [D, BS]
            "w_encT": w_encT,
            "w_decT": w_decT,
            "w_combT": w_combT,
            "w_ediag": w_ediag,
        })
    return in_maps


_NC_CACHE = {}


def get_nc():
    if "nc" not in _NC_CACHE:
        nc = bacc.Bacc("TRN2", debug=False, num_devices=NCORES)
        build_device_program(nc)
        nc.finalize()
        _NC_CACHE["nc"] = nc
    return _NC_CACHE["nc"]


def kernel(encoder_outputs, decoder_state, prev_attention_weights,
           W_enc, W_dec, conv_w, W_loc, W_e, b_e, _trace=False, _result_box=None):
    in_maps = host_prepare(
        encoder_outputs, decoder_state, prev_attention_weights,
        W_enc, W_dec, conv_w, W_loc, W_e, b_e,
    )
    nc = get_nc()
    res = bass_utils.run_bass_kernel_spmd(
        nc, in_maps, core_ids=list(range(NCORES)), trace=_trace,
    )
    if _result_box is not None:
        _result_box.append(res)
    ctx = np.concatenate([r["context_out"] for r in res.results], axis=0)
    attn = np.concatenate([r["attn_out"] for r in res.results], axis=0)
    return ctx.astype(np.float32), attn.astype(np.float32)
